# revision 70
# baseline (speedup 1.0000x reference)
"""Causal self-attention (B=4, T=4096, C=128) on 8 trn2 NeuronCores.

Sharding: core c -> (batch b=c//2, key-parity class h=c%2).
Each core processes ALL queries of its batch against the key chunks
j === h (mod 2) (128-wide chunks) -> exactly half the causal work per
core, identical instruction stream on every core (SPMD-uniform; only
the input DATA differs per core). Each core emits the unnormalized
partial attention output ou = w~^T V restricted to its key class and
the partial softmax denominators se; the host combines
  out[b] = (ou_h0 + ou_h1) / (se_h0 + se_h1).

Device math per query block (512 queries), all in "transposed score"
domain so no on-device transposes are needed (all matmuls are N=512
float32r, 1 cycle/row on the PE):
  Y^T  = matmul(lhsT=Wq^T Wk (host-fused), rhs=x^T)   [c, q]  (Y = Q Wk)
  S^T  = matmul(lhsT=xk^T chunk, rhs=Y^T)             [s, q]  (scores^T)
  w~   = exp(S^T / sqrt(C)) * causal_mask             [s, q]
  row  += matmul(lhsT=ones, rhs=w~ (chunk pairs       [1, q]  (sumexp)
          pre-summed on the vector engine))
  u    += matmul(lhsT=xk chunk, rhs=w~)               [c, q]  (Xk^T w~^T)
  ou^T = matmul(lhsT=Wv^T, rhs=u)                     [c, q]  (unnormalized)
"""

import math

import numpy as np

import concourse.mybir as mybir
import concourse.tile as tile
from concourse import bacc
from concourse.bass_utils import run_bass_kernel_spmd

B, T, C = 4, 4096, 128
P = 128            # partition width / head dim / key chunk
QB = 512           # query block (matmul free dim)
NQB = T // QB      # 8 query blocks
NCH = T // P // 2  # 16 key chunks per parity class

# dtype for matmul inputs (float32r = 4x matmul throughput vs float32)
MDT = mybir.dt.float32r

F32 = mybir.dt.float32


def build_kernel(cfg=None):
    base = dict(w_bufs=7, s_bufs=4, u_bufs=2, o_bufs=1, row_bufs=1)
    base.update(cfg or {})
    cfg = base
    nc = bacc.Bacc(None, target_bir_lowering=False)

    # Inputs (per-core data; identical shapes/names on every core).
    xT = nc.dram_tensor("xT", [P, T], MDT, kind="ExternalInput")      # x[b].T
    xkT = nc.dram_tensor("xkT", [P, NCH * P], MDT, kind="ExternalInput")
    xk = nc.dram_tensor("xk", [NCH * P, P], MDT, kind="ExternalInput")
    wqk = nc.dram_tensor("wqk", [P, P], MDT, kind="ExternalInput")    # Wq.T @ Wk
    wv_t = nc.dram_tensor("wv_t", [P, P], MDT, kind="ExternalInput")  # Wv.T
    mask_lo = nc.dram_tensor("mask_lo", [P, QB], MDT, kind="ExternalInput")
    mask_hi = nc.dram_tensor("mask_hi", [P, QB], MDT, kind="ExternalInput")
    ones = nc.dram_tensor("ones", [P, 1], MDT, kind="ExternalInput")

    # Outputs (ou is stored transposed: [C, T])
    ou = nc.dram_tensor("ou", [P, T], F32, kind="ExternalOutput")
    se = nc.dram_tensor("se", [NQB, QB], F32, kind="ExternalOutput")

    scale = 1.0 / math.sqrt(C)

    with tile.TileContext(nc) as tc:
        with (
            tc.tile_pool(name="const", bufs=1) as const,
            tc.tile_pool(name="wpool", bufs=cfg["w_bufs"]) as wpool,
            tc.tile_pool(name="upool", bufs=2) as upool,
            tc.tile_pool(name="wspool", bufs=2) as wspool,
            tc.tile_pool(name="opool", bufs=4) as opool,
            tc.tile_pool(name="spool", bufs=2) as spool,
            tc.tile_pool(name="ps_s", bufs=cfg["s_bufs"], space="PSUM") as ps_s,
            tc.tile_pool(name="ps_row", bufs=cfg["row_bufs"], space="PSUM") as ps_row,
            tc.tile_pool(name="ps_u", bufs=cfg["u_bufs"], space="PSUM") as ps_u,
            tc.tile_pool(name="ps_o", bufs=cfg["o_bufs"], space="PSUM") as ps_o,
        ):
            # ---- load constants / activations ----
            # Small constants first: the HWDGE generates descriptors in
            # issue order, so anything the first matmuls need must go first.
            wqk_sb = const.tile([P, P], MDT)
            wv_t_sb = const.tile([P, P], MDT)
            ml_sb = const.tile([P, QB], MDT)
            mh_sb = const.tile([P, QB], MDT)
            ones_sb = const.tile([P, 1], MDT)
            xT_sb = const.tile([P, T], MDT)
            xkT_sb = const.tile([P, NCH * P], MDT)
            xk_sb = const.tile([P, NCH * P], MDT)

            # DMA issue order == descriptor-generation order. The HWDGE is
            # ONE shared unit for the sync+scalar queues (~0.63us per
            # dma_start, serialized); SWDGE (gpsimd/Pool) is independent.
            # HWDGE: critical path first (wqk, xT7), then key-chunk groups
            # and remaining xT blocks in consumption order (qblocks 7->0).
            # SWDGE: ones, xk groups, masks, wv.
            nc.sync.dma_start(wqk_sb[:], wqk[:])
            nc.sync.dma_start(
                xT_sb[:, (NQB - 1) * QB :], xT[:, (NQB - 1) * QB :]
            )
            nc.sync.dma_start(
                xT_sb[:, (NQB - 2) * QB : (NQB - 1) * QB],
                xT[:, (NQB - 2) * QB : (NQB - 1) * QB],
            )
            nc.gpsimd.dma_start(ones_sb[:], ones[:])
            for g in range(0, NCH, 4):
                gs = slice(g * P, (g + 4) * P)
                nc.gpsimd.dma_start(
                    xk_sb[:, gs].rearrange("p (g c) -> p g c", g=4),
                    xk[gs, :].rearrange("(g p) c -> p g c", p=P),
                )
            nc.gpsimd.dma_start(ml_sb[:], mask_lo[:])
            nc.gpsimd.dma_start(mh_sb[:], mask_hi[:])
            nc.gpsimd.dma_start(wv_t_sb[:], wv_t[:])

            order = cfg.get("order") or [7, 6, 1, 5, 0, 4, 3, 2]
            gs0 = slice(0, 4 * P)
            nc.sync.dma_start(xkT_sb[:, gs0], xkT[:, gs0])
            xts = [n for n in order[1:] if n not in (NQB - 1, NQB - 2)]
            for g in range(4, NCH, 4):
                gs = slice(g * P, (g + 4) * P)
                nc.sync.dma_start(xkT_sb[:, gs], xkT[:, gs])
                if xts:
                    n = xts.pop(0)
                    nc.sync.dma_start(
                        xT_sb[:, n * QB : (n + 1) * QB],
                        xT[:, n * QB : (n + 1) * QB],
                    )
            for n in xts:
                nc.sync.dma_start(
                    xT_sb[:, n * QB : (n + 1) * QB], xT[:, n * QB : (n + 1) * QB]
                )

            # ---- attention per query block ----
            # Per-qblock head (Y^T projection) and epilogue (u/se
            # evacuation + Wv projection) are interleaved into the
            # surrounding qblocks' chunk streams so the PE keeps busy.
            y_all = const.tile([P, T], MDT)

            def emit_head(i):
                qs = slice(i * QB, (i + 1) * QB)
                ps = ps_s.tile([P, QB], F32, tag="ps")
                nc.tensor.matmul(ps[:], wqk_sb[:], xT_sb[:, qs], start=True, stop=True)
                nc.vector.tensor_copy(out=y_all[:, qs], in_=ps[:])

            def make_tail(i, psu, psr, final=False):
                def tail():
                    qs = slice(i * QB, (i + 1) * QB)
                    se_sb = spool.tile([1, QB], F32)
                    nc.vector.tensor_copy(out=se_sb[:], in_=psr[:])
                    nc.sync.dma_start(se[i : i + 1, :], se_sb[:])
                    u_sb = upool.tile([P, QB], MDT)
                    pso = ps_o.tile([P, QB], F32)
                    o_sb = opool.tile([P, QB], F32)
                    if not final:
                        nc.vector.tensor_copy(out=u_sb[:], in_=psu[:])
                        nc.tensor.matmul(
                            pso[:], wv_t_sb[:], u_sb[:], start=True, stop=True
                        )
                        nc.vector.tensor_copy(out=o_sb[:], in_=pso[:])
                        nc.sync.dma_start(ou[:, qs], o_sb[:])
                    else:
                        # Final epilogue: nothing left to hide behind, so
                        # pipeline it in half-width pieces across queues.
                        H = QB // 2
                        for k in range(2):
                            hs = slice(k * H, (k + 1) * H)
                            ds = slice(i * QB + k * H, i * QB + (k + 1) * H)
                            nc.vector.tensor_copy(out=u_sb[:, hs], in_=psu[:, hs])
                            nc.tensor.matmul(
                                pso[:, hs], wv_t_sb[:], u_sb[:, hs],
                                start=True, stop=True,
                            )
                            nc.vector.tensor_copy(out=o_sb[:, hs], in_=pso[:, hs])
                            q_eng = nc.sync if k == 0 else nc.scalar
                            q_eng.dma_start(ou[:, ds], o_sb[:, hs])

                return tail

            def emit_last_accum(psu_t, psr_t, nch_, wt):
                # accum for a qblock's final (restricted) chunk; explicit
                # args because the loop locals are rebound across qblocks
                c = nch_ - 1
                cs = slice(c * P, (c + 1) * P)
                nc.tensor.matmul(
                    psr_t[:, 256:], ones_sb[:], wt[:, 256:],
                    start=False, stop=True,
                )
                nc.tensor.matmul(
                    psu_t[:, 256:], xk_sb[:, cs], wt[:, 256:],
                    start=False, stop=True,
                )

            pending_tail = None
            pending_accum = None
            heads = list(order)
            emit_head(heads.pop(0))
            emit_head(heads.pop(0))
            for oi, i in enumerate(order):
                nch = 2 * (i + 1)
                ysb = y_all[:, i * QB : (i + 1) * QB]

                psu = ps_u.tile([P, QB], F32)
                psr = ps_row.tile([1, QB], F32)

                def emit_score(c):
                    # Final (diagonal) chunk: queries < 256 are entirely
                    # before this key chunk for both parities -> compute
                    # only columns [256, 512).
                    o = 256 if c == nch - 1 else 0
                    cs = slice(c * P, (c + 1) * P)
                    pss = ps_s.tile([P, QB], F32, tag="ps")
                    nc.tensor.matmul(
                        pss[:, o:], xkT_sb[:, cs], ysb[:, o:], start=True, stop=True
                    )
                    wt = wpool.tile([P, QB], MDT)
                    nc.scalar.activation(
                        wt[:, o:], pss[:, o:], mybir.ActivationFunctionType.Exp,
                        scale=scale,
                    )
                    if c == nch - 2:
                        nc.vector.tensor_mul(
                            out=wt[:, 0:256], in0=wt[:, 0:256], in1=ml_sb[:, 0:256]
                        )
                    elif c == nch - 1:
                        nc.vector.tensor_mul(
                            out=wt[:, 256:], in0=wt[:, 256:], in1=mh_sb[:, 256:]
                        )
                    return wt

                w_stash = []

                def emit_accum(c, wt):
                    o = 256 if c == nch - 1 else 0
                    cs = slice(c * P, (c + 1) * P)
                    first, last = c == 0, c == nch - 1
                    # psr (sumexp) uses the same lhsT for every chunk, so
                    # chunk pairs are pre-summed on DVE and streamed
                    # through the PE once. The final two chunks (mask /
                    # restricted columns) stay individual.
                    if c < nch - 2:
                        if not w_stash:
                            w_stash.append((c, wt))
                        else:
                            c0, wt0 = w_stash.pop()
                            ws = wspool.tile([P, QB], MDT)
                            nc.vector.tensor_add(out=ws[:], in0=wt0[:], in1=wt[:])
                            nc.tensor.matmul(
                                psr[:], ones_sb[:], ws[:],
                                start=(c0 == 0), stop=False,
                            )
                    else:
                        nc.tensor.matmul(
                            psr[:, o:], ones_sb[:], wt[:, o:],
                            start=first, stop=last,
                        )
                    nc.tensor.matmul(
                        psu[:, o:], xk_sb[:, cs], wt[:, o:], start=first, stop=last
                    )

                # software-pipeline by one chunk; the previous qblock's
                # LAST accum, its epilogue, and the next qblock's head are
                # all deferred into this qblock's chunk stream so the PE
                # never waits on the exp->mask chain at a boundary.
                wt_prev = emit_score(0)
                if pending_accum is not None:
                    pending_accum()
                    pending_accum = None
                for c in range(1, nch):
                    wt_c = emit_score(c)
                    emit_accum(c - 1, wt_prev)
                    wt_prev = wt_c
                    if c == 1 and pending_tail is not None:
                        pending_tail()
                        pending_tail = None
                    c_head = 1 if nch == 2 else max(2, nch - 4)
                    if c == c_head and heads:
                        emit_head(heads.pop(0))
                pending_accum = (
                    lambda pu=psu, pr=psr, n=nch, w=wt_prev: emit_last_accum(
                        pu, pr, n, w
                    )
                )
                if pending_tail is not None:  # nch == 2 case
                    pending_tail()
                pending_tail = make_tail(i, psu, psr, final=oi == NQB - 1)
            pending_accum()
            pending_tail()

    nc.compile()
    return nc


_NC_CACHE = {}


def _get_nc():
    if "nc" not in _NC_CACHE:
        _NC_CACHE["nc"] = build_kernel()
    return _NC_CACHE["nc"]


def _core_inputs(xb, Wq, Wk, Wv, h):
    """Build the input map for one core (batch data xb [T,C], parity h)."""
    rows = np.concatenate(
        [np.arange(j * P, (j + 1) * P) for j in range(h, T // P, 2)]
    )
    xk = np.ascontiguousarray(xb[rows])            # [NCH*P, C]
    s = np.arange(P)[:, None]
    q = np.arange(QB)[None, :]
    mask_lo = (q >= s + P * h).astype(np.float32)
    mask_hi = (q >= s + P * (h + 2)).astype(np.float32)
    return {
        "xT": np.ascontiguousarray(xb.T),
        "xkT": np.ascontiguousarray(xk.T),
        "xk": xk,
        "wqk": np.ascontiguousarray(Wq.T @ Wk),
        "wv_t": np.ascontiguousarray(Wv.T),
        "mask_lo": mask_lo,
        "mask_hi": mask_hi,
        "ones": np.ones((P, 1), dtype=np.float32),
    }


def kernel(x, Wq, Wk, Wv, _trace=False):
    x = np.asarray(x, dtype=np.float32)
    Wq = np.asarray(Wq, dtype=np.float32)
    Wk = np.asarray(Wk, dtype=np.float32)
    Wv = np.asarray(Wv, dtype=np.float32)

    nc = _get_nc()
    in_maps = [_core_inputs(x[c // 2], Wq, Wk, Wv, c % 2) for c in range(8)]
    try:
        res = run_bass_kernel_spmd(
            nc, in_maps, core_ids=list(range(8)), trace=_trace
        )
    except ModuleNotFoundError:
        # axon NTFF profiling hook unavailable in this container
        res = run_bass_kernel_spmd(nc, in_maps, core_ids=list(range(8)))
    if _trace:
        _NC_CACHE["last_results"] = res

    out = np.empty((B, T, C), dtype=np.float32)
    for b in range(B):
        a, bb = res.results[2 * b], res.results[2 * b + 1]
        denom = a["se"].reshape(T) + bb["se"].reshape(T)
        out[b] = ((a["ou"] + bb["ou"]) / denom[None, :]).T
    return out


# revision 71
# speedup vs baseline: 1.0152x; 1.0152x over previous
"""Causal self-attention (B=4, T=4096, C=128) on 8 trn2 NeuronCores.

Sharding: core c -> (batch b=c//2, key-parity class h=c%2).
Each core processes ALL queries of its batch against the key chunks
j === h (mod 2) (128-wide chunks) -> exactly half the causal work per
core, identical instruction stream on every core (SPMD-uniform; only
the input DATA differs per core). Each core emits the unnormalized
partial attention output ou = w~^T V restricted to its key class and
the partial softmax denominators se; the host combines
  out[b] = (ou_h0 + ou_h1) / (se_h0 + se_h1).

Device math per query block (512 queries), all in "transposed score"
domain so no on-device transposes are needed (all matmuls are N=512
float32r, 1 cycle/row on the PE):
  Y^T  = matmul(lhsT=Wq^T Wk (host-fused), rhs=x^T)   [c, q]  (Y = Q Wk)
  S^T  = matmul(lhsT=xk^T chunk, rhs=Y^T)             [s, q]  (scores^T)
  w~   = exp(S^T / sqrt(C)) * causal_mask             [s, q]
  row  += matmul(lhsT=ones, rhs=w~ (chunk pairs       [1, q]  (sumexp)
          pre-summed on the vector engine))
  u    += matmul(lhsT=xk chunk, rhs=w~)               [c, q]  (Xk^T w~^T)
  ou^T = matmul(lhsT=Wv^T, rhs=u)                     [c, q]  (unnormalized)
"""

import math

import numpy as np

import concourse.mybir as mybir
import concourse.tile as tile
from concourse import bacc
from concourse.bass_utils import run_bass_kernel_spmd

B, T, C = 4, 4096, 128
P = 128            # partition width / head dim / key chunk
QB = 512           # query block (matmul free dim)
NQB = T // QB      # 8 query blocks
NCH = T // P // 2  # 16 key chunks per parity class

# dtype for matmul inputs (float32r = 4x matmul throughput vs float32)
MDT = mybir.dt.float32r

F32 = mybir.dt.float32


def build_kernel(cfg=None):
    base = dict(w_bufs=7, s_bufs=4, u_bufs=2, o_bufs=1, row_bufs=1)
    base.update(cfg or {})
    cfg = base
    nc = bacc.Bacc(None, target_bir_lowering=False)

    # Inputs (per-core data; identical shapes/names on every core).
    xT = nc.dram_tensor("xT", [P, T], MDT, kind="ExternalInput")      # x[b].T
    xkT = nc.dram_tensor("xkT", [P, NCH * P], MDT, kind="ExternalInput")
    xk = nc.dram_tensor("xk", [NCH * P, P], MDT, kind="ExternalInput")
    wqk = nc.dram_tensor("wqk", [P, P], MDT, kind="ExternalInput")    # Wq.T @ Wk
    wv_t = nc.dram_tensor("wv_t", [P, P], MDT, kind="ExternalInput")  # Wv.T
    mask_lo = nc.dram_tensor("mask_lo", [P, QB], MDT, kind="ExternalInput")
    mask_hi = nc.dram_tensor("mask_hi", [P, QB], MDT, kind="ExternalInput")
    ones = nc.dram_tensor("ones", [P, 1], MDT, kind="ExternalInput")

    # Outputs (ou is stored transposed: [C, T])
    ou = nc.dram_tensor("ou", [P, T], F32, kind="ExternalOutput")
    se = nc.dram_tensor("se", [NQB, QB], F32, kind="ExternalOutput")

    scale = 1.0 / math.sqrt(C)

    with tile.TileContext(nc) as tc:
        with (
            tc.tile_pool(name="const", bufs=1) as const,
            tc.tile_pool(name="wpool", bufs=cfg["w_bufs"]) as wpool,
            tc.tile_pool(name="upool", bufs=2) as upool,
            tc.tile_pool(name="wspool", bufs=2) as wspool,
            tc.tile_pool(name="opool", bufs=4) as opool,
            tc.tile_pool(name="spool", bufs=2) as spool,
            tc.tile_pool(name="ps_s", bufs=cfg["s_bufs"], space="PSUM") as ps_s,
            tc.tile_pool(name="ps_row", bufs=cfg["row_bufs"], space="PSUM") as ps_row,
            tc.tile_pool(name="ps_u", bufs=cfg["u_bufs"], space="PSUM") as ps_u,
            tc.tile_pool(name="ps_o", bufs=cfg["o_bufs"], space="PSUM") as ps_o,
        ):
            # ---- load constants / activations ----
            # Small constants first: the HWDGE generates descriptors in
            # issue order, so anything the first matmuls need must go first.
            wqk_sb = const.tile([P, P], MDT)
            wv_t_sb = const.tile([P, P], MDT)
            ml_sb = const.tile([P, QB], MDT)
            mh_sb = const.tile([P, QB], MDT)
            ones_sb = const.tile([P, 1], MDT)
            xT_sb = const.tile([P, T], MDT)
            xkT_sb = const.tile([P, NCH * P], MDT)
            xk_sb = const.tile([P, NCH * P], MDT)

            # DMA issue order == descriptor-generation order. The HWDGE is
            # ONE shared unit for the sync+scalar queues (~0.63us per
            # dma_start, serialized); SWDGE (gpsimd/Pool) is independent.
            # HWDGE: critical path first (wqk, xT7), then key-chunk groups
            # and remaining xT blocks in consumption order (qblocks 7->0).
            # SWDGE: ones, xk groups, masks, wv.
            nc.sync.dma_start(wqk_sb[:], wqk[:])
            nc.gpsimd.dma_start(
                xT_sb[:, (NQB - 1) * QB :], xT[:, (NQB - 1) * QB :]
            )
            nc.sync.dma_start(
                xT_sb[:, (NQB - 2) * QB : (NQB - 1) * QB],
                xT[:, (NQB - 2) * QB : (NQB - 1) * QB],
            )
            nc.gpsimd.dma_start(ones_sb[:], ones[:])
            for g in range(0, NCH, 4):
                gs = slice(g * P, (g + 4) * P)
                nc.gpsimd.dma_start(
                    xk_sb[:, gs].rearrange("p (g c) -> p g c", g=4),
                    xk[gs, :].rearrange("(g p) c -> p g c", p=P),
                )
            nc.gpsimd.dma_start(ml_sb[:], mask_lo[:])
            nc.gpsimd.dma_start(mh_sb[:], mask_hi[:])
            nc.gpsimd.dma_start(wv_t_sb[:], wv_t[:])

            order = cfg.get("order") or [7, 6, 1, 5, 0, 4, 3, 2]
            gs0 = slice(0, 4 * P)
            nc.sync.dma_start(xkT_sb[:, gs0], xkT[:, gs0])
            xts = [n for n in order[1:] if n not in (NQB - 1, NQB - 2)]
            for g in range(4, NCH, 4):
                gs = slice(g * P, (g + 4) * P)
                nc.sync.dma_start(xkT_sb[:, gs], xkT[:, gs])
                if xts:
                    n = xts.pop(0)
                    nc.sync.dma_start(
                        xT_sb[:, n * QB : (n + 1) * QB],
                        xT[:, n * QB : (n + 1) * QB],
                    )
            for n in xts:
                nc.sync.dma_start(
                    xT_sb[:, n * QB : (n + 1) * QB], xT[:, n * QB : (n + 1) * QB]
                )

            # ---- attention per query block ----
            # Per-qblock head (Y^T projection) and epilogue (u/se
            # evacuation + Wv projection) are interleaved into the
            # surrounding qblocks' chunk streams so the PE keeps busy.
            y_all = const.tile([P, T], MDT)

            def emit_head(i):
                qs = slice(i * QB, (i + 1) * QB)
                ps = ps_s.tile([P, QB], F32, tag="ps")
                nc.tensor.matmul(ps[:], wqk_sb[:], xT_sb[:, qs], start=True, stop=True)
                nc.vector.tensor_copy(out=y_all[:, qs], in_=ps[:])

            def make_tail(i, psu, psr, final=False):
                def tail():
                    qs = slice(i * QB, (i + 1) * QB)
                    se_sb = spool.tile([1, QB], F32)
                    nc.vector.tensor_copy(out=se_sb[:], in_=psr[:])
                    nc.sync.dma_start(se[i : i + 1, :], se_sb[:])
                    u_sb = upool.tile([P, QB], MDT)
                    pso = ps_o.tile([P, QB], F32)
                    o_sb = opool.tile([P, QB], F32)
                    if not final:
                        nc.vector.tensor_copy(out=u_sb[:], in_=psu[:])
                        nc.tensor.matmul(
                            pso[:], wv_t_sb[:], u_sb[:], start=True, stop=True
                        )
                        nc.vector.tensor_copy(out=o_sb[:], in_=pso[:])
                        nc.sync.dma_start(ou[:, qs], o_sb[:])
                    else:
                        # Final epilogue: nothing left to hide behind, so
                        # pipeline it in half-width pieces across queues.
                        H = QB // 2
                        for k in range(2):
                            hs = slice(k * H, (k + 1) * H)
                            ds = slice(i * QB + k * H, i * QB + (k + 1) * H)
                            nc.vector.tensor_copy(out=u_sb[:, hs], in_=psu[:, hs])
                            nc.tensor.matmul(
                                pso[:, hs], wv_t_sb[:], u_sb[:, hs],
                                start=True, stop=True,
                            )
                            nc.vector.tensor_copy(out=o_sb[:, hs], in_=pso[:, hs])
                            q_eng = nc.sync if k == 0 else nc.scalar
                            q_eng.dma_start(ou[:, ds], o_sb[:, hs])

                return tail

            def emit_last_accum(psu_t, psr_t, nch_, wt):
                # accum for a qblock's final (restricted) chunk; explicit
                # args because the loop locals are rebound across qblocks
                c = nch_ - 1
                cs = slice(c * P, (c + 1) * P)
                nc.tensor.matmul(
                    psr_t[:, 256:], ones_sb[:], wt[:, 256:],
                    start=False, stop=True,
                )
                nc.tensor.matmul(
                    psu_t[:, 256:], xk_sb[:, cs], wt[:, 256:],
                    start=False, stop=True,
                )

            pending_tail = None
            pending_accum = None
            heads = list(order)
            emit_head(heads.pop(0))
            emit_head(heads.pop(0))
            for oi, i in enumerate(order):
                nch = 2 * (i + 1)
                ysb = y_all[:, i * QB : (i + 1) * QB]

                psu = ps_u.tile([P, QB], F32)
                psr = ps_row.tile([1, QB], F32)

                def emit_score(c):
                    # Final (diagonal) chunk: queries < 256 are entirely
                    # before this key chunk for both parities -> compute
                    # only columns [256, 512).
                    o = 256 if c == nch - 1 else 0
                    cs = slice(c * P, (c + 1) * P)
                    pss = ps_s.tile([P, QB], F32, tag="ps")
                    nc.tensor.matmul(
                        pss[:, o:], xkT_sb[:, cs], ysb[:, o:], start=True, stop=True
                    )
                    wt = wpool.tile([P, QB], MDT)
                    nc.scalar.activation(
                        wt[:, o:], pss[:, o:], mybir.ActivationFunctionType.Exp,
                        scale=scale,
                    )
                    if c == nch - 2:
                        nc.vector.tensor_mul(
                            out=wt[:, 0:256], in0=wt[:, 0:256], in1=ml_sb[:, 0:256]
                        )
                    elif c == nch - 1:
                        nc.vector.tensor_mul(
                            out=wt[:, 256:], in0=wt[:, 256:], in1=mh_sb[:, 256:]
                        )
                    return wt

                w_stash = []

                def emit_accum(c, wt):
                    o = 256 if c == nch - 1 else 0
                    cs = slice(c * P, (c + 1) * P)
                    first, last = c == 0, c == nch - 1
                    # psr (sumexp) uses the same lhsT for every chunk, so
                    # chunk pairs are pre-summed on DVE and streamed
                    # through the PE once. The final two chunks (mask /
                    # restricted columns) stay individual.
                    if c < nch - 2:
                        if not w_stash:
                            w_stash.append((c, wt))
                        else:
                            c0, wt0 = w_stash.pop()
                            ws = wspool.tile([P, QB], MDT)
                            nc.vector.tensor_add(out=ws[:], in0=wt0[:], in1=wt[:])
                            nc.tensor.matmul(
                                psr[:], ones_sb[:], ws[:],
                                start=(c0 == 0), stop=False,
                            )
                    else:
                        nc.tensor.matmul(
                            psr[:, o:], ones_sb[:], wt[:, o:],
                            start=first, stop=last,
                        )
                    nc.tensor.matmul(
                        psu[:, o:], xk_sb[:, cs], wt[:, o:], start=first, stop=last
                    )

                # software-pipeline by one chunk; the previous qblock's
                # LAST accum, its epilogue, and the next qblock's head are
                # all deferred into this qblock's chunk stream so the PE
                # never waits on the exp->mask chain at a boundary.
                wt_prev = emit_score(0)
                if pending_accum is not None:
                    pending_accum()
                    pending_accum = None
                for c in range(1, nch):
                    wt_c = emit_score(c)
                    emit_accum(c - 1, wt_prev)
                    wt_prev = wt_c
                    if c == 1 and pending_tail is not None:
                        pending_tail()
                        pending_tail = None
                    c_head = 1 if nch == 2 else max(2, nch - 4)
                    if c == c_head and heads:
                        emit_head(heads.pop(0))
                pending_accum = (
                    lambda pu=psu, pr=psr, n=nch, w=wt_prev: emit_last_accum(
                        pu, pr, n, w
                    )
                )
                if pending_tail is not None:  # nch == 2 case
                    pending_tail()
                pending_tail = make_tail(i, psu, psr, final=oi == NQB - 1)
            pending_accum()
            pending_tail()

    nc.compile()
    return nc


_NC_CACHE = {}


def _get_nc():
    if "nc" not in _NC_CACHE:
        _NC_CACHE["nc"] = build_kernel()
    return _NC_CACHE["nc"]


def _core_inputs(xb, Wq, Wk, Wv, h):
    """Build the input map for one core (batch data xb [T,C], parity h)."""
    rows = np.concatenate(
        [np.arange(j * P, (j + 1) * P) for j in range(h, T // P, 2)]
    )
    xk = np.ascontiguousarray(xb[rows])            # [NCH*P, C]
    s = np.arange(P)[:, None]
    q = np.arange(QB)[None, :]
    mask_lo = (q >= s + P * h).astype(np.float32)
    mask_hi = (q >= s + P * (h + 2)).astype(np.float32)
    return {
        "xT": np.ascontiguousarray(xb.T),
        "xkT": np.ascontiguousarray(xk.T),
        "xk": xk,
        "wqk": np.ascontiguousarray(Wq.T @ Wk),
        "wv_t": np.ascontiguousarray(Wv.T),
        "mask_lo": mask_lo,
        "mask_hi": mask_hi,
        "ones": np.ones((P, 1), dtype=np.float32),
    }


def kernel(x, Wq, Wk, Wv, _trace=False):
    x = np.asarray(x, dtype=np.float32)
    Wq = np.asarray(Wq, dtype=np.float32)
    Wk = np.asarray(Wk, dtype=np.float32)
    Wv = np.asarray(Wv, dtype=np.float32)

    nc = _get_nc()
    in_maps = [_core_inputs(x[c // 2], Wq, Wk, Wv, c % 2) for c in range(8)]
    try:
        res = run_bass_kernel_spmd(
            nc, in_maps, core_ids=list(range(8)), trace=_trace
        )
    except ModuleNotFoundError:
        # axon NTFF profiling hook unavailable in this container
        res = run_bass_kernel_spmd(nc, in_maps, core_ids=list(range(8)))
    if _trace:
        _NC_CACHE["last_results"] = res

    out = np.empty((B, T, C), dtype=np.float32)
    for b in range(B):
        a, bb = res.results[2 * b], res.results[2 * b + 1]
        denom = a["se"].reshape(T) + bb["se"].reshape(T)
        out[b] = ((a["ou"] + bb["ou"]) / denom[None, :]).T
    return out


# revision 72
# speedup vs baseline: 1.0191x; 1.0039x over previous
"""Causal self-attention (B=4, T=4096, C=128) on 8 trn2 NeuronCores.

Sharding: core c -> (batch b=c//2, key-parity class h=c%2).
Each core processes ALL queries of its batch against the key chunks
j === h (mod 2) (128-wide chunks) -> exactly half the causal work per
core, identical instruction stream on every core (SPMD-uniform; only
the input DATA differs per core). Each core emits the unnormalized
partial attention output ou = w~^T V restricted to its key class and
the partial softmax denominators se; the host combines
  out[b] = (ou_h0 + ou_h1) / (se_h0 + se_h1).

Device math per query block (512 queries), all in "transposed score"
domain so no on-device transposes are needed (all matmuls are N=512
float32r, 1 cycle/row on the PE):
  Y^T  = matmul(lhsT=Wq^T Wk (host-fused), rhs=x^T)   [c, q]  (Y = Q Wk)
  S^T  = matmul(lhsT=xk^T chunk, rhs=Y^T)             [s, q]  (scores^T)
  w~   = exp(S^T / sqrt(C)) * causal_mask             [s, q]
  row  += matmul(lhsT=ones, rhs=w~ (chunk pairs       [1, q]  (sumexp)
          pre-summed on the vector engine))
  u    += matmul(lhsT=xk chunk, rhs=w~)               [c, q]  (Xk^T w~^T)
  ou^T = matmul(lhsT=Wv^T, rhs=u)                     [c, q]  (unnormalized)
"""

import math

import numpy as np

import concourse.mybir as mybir
import concourse.tile as tile
from concourse import bacc
from concourse.bass_utils import run_bass_kernel_spmd

B, T, C = 4, 4096, 128
P = 128            # partition width / head dim / key chunk
QB = 512           # query block (matmul free dim)
NQB = T // QB      # 8 query blocks
NCH = T // P // 2  # 16 key chunks per parity class

# dtype for matmul inputs (float32r = 4x matmul throughput vs float32)
MDT = mybir.dt.float32r

F32 = mybir.dt.float32


def build_kernel(cfg=None):
    base = dict(w_bufs=9, s_bufs=4, u_bufs=2, o_bufs=1, row_bufs=1)
    base.update(cfg or {})
    cfg = base
    nc = bacc.Bacc(None, target_bir_lowering=False)

    # Inputs (per-core data; identical shapes/names on every core).
    xT = nc.dram_tensor("xT", [P, T], MDT, kind="ExternalInput")      # x[b].T
    xkT = nc.dram_tensor("xkT", [P, NCH * P], MDT, kind="ExternalInput")
    xk = nc.dram_tensor("xk", [NCH * P, P], MDT, kind="ExternalInput")
    wqk = nc.dram_tensor("wqk", [P, P], MDT, kind="ExternalInput")    # Wq.T @ Wk
    wv_t = nc.dram_tensor("wv_t", [P, P], MDT, kind="ExternalInput")  # Wv.T
    mask_lo = nc.dram_tensor("mask_lo", [P, QB], MDT, kind="ExternalInput")
    mask_hi = nc.dram_tensor("mask_hi", [P, QB], MDT, kind="ExternalInput")
    ones = nc.dram_tensor("ones", [P, 1], MDT, kind="ExternalInput")

    # Outputs (ou is stored transposed: [C, T])
    ou = nc.dram_tensor("ou", [P, T], F32, kind="ExternalOutput")
    se = nc.dram_tensor("se", [NQB, QB], F32, kind="ExternalOutput")

    scale = 1.0 / math.sqrt(C)

    with tile.TileContext(nc) as tc:
        with (
            tc.tile_pool(name="const", bufs=1) as const,
            tc.tile_pool(name="wpool", bufs=cfg["w_bufs"]) as wpool,
            tc.tile_pool(name="upool", bufs=2) as upool,
            tc.tile_pool(name="wspool", bufs=2) as wspool,
            tc.tile_pool(name="opool", bufs=4) as opool,
            tc.tile_pool(name="spool", bufs=2) as spool,
            tc.tile_pool(name="ps_s", bufs=cfg["s_bufs"], space="PSUM") as ps_s,
            tc.tile_pool(name="ps_row", bufs=cfg["row_bufs"], space="PSUM") as ps_row,
            tc.tile_pool(name="ps_u", bufs=cfg["u_bufs"], space="PSUM") as ps_u,
            tc.tile_pool(name="ps_o", bufs=cfg["o_bufs"], space="PSUM") as ps_o,
        ):
            # ---- load constants / activations ----
            # Small constants first: the HWDGE generates descriptors in
            # issue order, so anything the first matmuls need must go first.
            wqk_sb = const.tile([P, P], MDT)
            wv_t_sb = const.tile([P, P], MDT)
            ml_sb = const.tile([P, QB], MDT)
            mh_sb = const.tile([P, QB], MDT)
            ones_sb = const.tile([P, 1], MDT)
            xT_sb = const.tile([P, T], MDT)
            xkT_sb = const.tile([P, NCH * P], MDT)
            xk_sb = const.tile([P, NCH * P], MDT)

            # DMA issue order == descriptor-generation order. The HWDGE is
            # ONE shared unit for the sync+scalar queues (~0.63us per
            # dma_start, serialized); SWDGE (gpsimd/Pool) is independent.
            # HWDGE: critical path first (wqk, xT7), then key-chunk groups
            # and remaining xT blocks in consumption order (qblocks 7->0).
            # SWDGE: ones, xk groups, masks, wv.
            nc.sync.dma_start(wqk_sb[:], wqk[:])
            nc.gpsimd.dma_start(
                xT_sb[:, (NQB - 1) * QB :], xT[:, (NQB - 1) * QB :]
            )
            nc.sync.dma_start(
                xT_sb[:, (NQB - 2) * QB : (NQB - 1) * QB],
                xT[:, (NQB - 2) * QB : (NQB - 1) * QB],
            )
            nc.gpsimd.dma_start(ones_sb[:], ones[:])
            for g in range(0, NCH, 4):
                gs = slice(g * P, (g + 4) * P)
                nc.gpsimd.dma_start(
                    xk_sb[:, gs].rearrange("p (g c) -> p g c", g=4),
                    xk[gs, :].rearrange("(g p) c -> p g c", p=P),
                )
            nc.gpsimd.dma_start(ml_sb[:], mask_lo[:])
            nc.gpsimd.dma_start(mh_sb[:], mask_hi[:])
            nc.gpsimd.dma_start(wv_t_sb[:], wv_t[:])

            order = cfg.get("order") or [7, 6, 1, 5, 0, 4, 3, 2]
            gs0 = slice(0, 4 * P)
            nc.sync.dma_start(xkT_sb[:, gs0], xkT[:, gs0])
            xts = [n for n in order[1:] if n not in (NQB - 1, NQB - 2)]
            for g in range(4, NCH, 4):
                gs = slice(g * P, (g + 4) * P)
                nc.sync.dma_start(xkT_sb[:, gs], xkT[:, gs])
                if xts:
                    n = xts.pop(0)
                    nc.sync.dma_start(
                        xT_sb[:, n * QB : (n + 1) * QB],
                        xT[:, n * QB : (n + 1) * QB],
                    )
            for n in xts:
                nc.sync.dma_start(
                    xT_sb[:, n * QB : (n + 1) * QB], xT[:, n * QB : (n + 1) * QB]
                )

            # ---- attention per query block ----
            # Per-qblock head (Y^T projection) and epilogue (u/se
            # evacuation + Wv projection) are interleaved into the
            # surrounding qblocks' chunk streams so the PE keeps busy.
            y_all = const.tile([P, T], MDT)

            def emit_head(i):
                qs = slice(i * QB, (i + 1) * QB)
                ps = ps_s.tile([P, QB], F32, tag="ps")
                nc.tensor.matmul(ps[:], wqk_sb[:], xT_sb[:, qs], start=True, stop=True)
                nc.vector.tensor_copy(out=y_all[:, qs], in_=ps[:])

            def make_tail(i, psu, psr, final=False):
                def tail():
                    qs = slice(i * QB, (i + 1) * QB)
                    se_sb = spool.tile([1, QB], F32)
                    nc.vector.tensor_copy(out=se_sb[:], in_=psr[:])
                    nc.sync.dma_start(se[i : i + 1, :], se_sb[:])
                    u_sb = upool.tile([P, QB], MDT)
                    pso = ps_o.tile([P, QB], F32)
                    o_sb = opool.tile([P, QB], F32)
                    if not final:
                        nc.vector.tensor_copy(out=u_sb[:], in_=psu[:])
                        nc.tensor.matmul(
                            pso[:], wv_t_sb[:], u_sb[:], start=True, stop=True
                        )
                        nc.vector.tensor_copy(out=o_sb[:], in_=pso[:])
                        nc.sync.dma_start(ou[:, qs], o_sb[:])
                    else:
                        # Final epilogue: nothing left to hide behind, so
                        # pipeline it in half-width pieces across queues.
                        H = QB // 2
                        for k in range(2):
                            hs = slice(k * H, (k + 1) * H)
                            ds = slice(i * QB + k * H, i * QB + (k + 1) * H)
                            nc.vector.tensor_copy(out=u_sb[:, hs], in_=psu[:, hs])
                            nc.tensor.matmul(
                                pso[:, hs], wv_t_sb[:], u_sb[:, hs],
                                start=True, stop=True,
                            )
                            nc.vector.tensor_copy(out=o_sb[:, hs], in_=pso[:, hs])
                            q_eng = nc.sync if k == 0 else nc.scalar
                            q_eng.dma_start(ou[:, ds], o_sb[:, hs])

                return tail

            def emit_last_accum(psu_t, psr_t, nch_, wt):
                # accum for a qblock's final (restricted) chunk; explicit
                # args because the loop locals are rebound across qblocks
                c = nch_ - 1
                cs = slice(c * P, (c + 1) * P)
                nc.tensor.matmul(
                    psr_t[:, 256:], ones_sb[:], wt[:, 256:],
                    start=False, stop=True,
                )
                nc.tensor.matmul(
                    psu_t[:, 256:], xk_sb[:, cs], wt[:, 256:],
                    start=False, stop=True,
                )

            pending_tail = None
            pending_accum = None
            heads = list(order)
            emit_head(heads.pop(0))
            emit_head(heads.pop(0))
            for oi, i in enumerate(order):
                nch = 2 * (i + 1)
                ysb = y_all[:, i * QB : (i + 1) * QB]

                psu = ps_u.tile([P, QB], F32)
                psr = ps_row.tile([1, QB], F32)

                def emit_score(c):
                    # Final (diagonal) chunk: queries < 256 are entirely
                    # before this key chunk for both parities -> compute
                    # only columns [256, 512).
                    o = 256 if c == nch - 1 else 0
                    cs = slice(c * P, (c + 1) * P)
                    pss = ps_s.tile([P, QB], F32, tag="ps")
                    nc.tensor.matmul(
                        pss[:, o:], xkT_sb[:, cs], ysb[:, o:], start=True, stop=True
                    )
                    wt = wpool.tile([P, QB], MDT)
                    nc.scalar.activation(
                        wt[:, o:], pss[:, o:], mybir.ActivationFunctionType.Exp,
                        scale=scale,
                    )
                    if c == nch - 2:
                        nc.vector.tensor_mul(
                            out=wt[:, 0:256], in0=wt[:, 0:256], in1=ml_sb[:, 0:256]
                        )
                    elif c == nch - 1:
                        nc.vector.tensor_mul(
                            out=wt[:, 256:], in0=wt[:, 256:], in1=mh_sb[:, 256:]
                        )
                    return wt

                w_stash = []

                def emit_accum(c, wt):
                    o = 256 if c == nch - 1 else 0
                    cs = slice(c * P, (c + 1) * P)
                    first, last = c == 0, c == nch - 1
                    # psr (sumexp) uses the same lhsT for every chunk, so
                    # chunk pairs are pre-summed on DVE and streamed
                    # through the PE once. The final two chunks (mask /
                    # restricted columns) stay individual.
                    if c < nch - 2:
                        if not w_stash:
                            w_stash.append((c, wt))
                        else:
                            c0, wt0 = w_stash.pop()
                            ws = wspool.tile([P, QB], MDT)
                            nc.vector.tensor_add(out=ws[:], in0=wt0[:], in1=wt[:])
                            nc.tensor.matmul(
                                psr[:], ones_sb[:], ws[:],
                                start=(c0 == 0), stop=False,
                            )
                    else:
                        nc.tensor.matmul(
                            psr[:, o:], ones_sb[:], wt[:, o:],
                            start=first, stop=last,
                        )
                    nc.tensor.matmul(
                        psu[:, o:], xk_sb[:, cs], wt[:, o:], start=first, stop=last
                    )

                # software-pipeline by one chunk; the previous qblock's
                # LAST accum, its epilogue, and the next qblock's head are
                # all deferred into this qblock's chunk stream so the PE
                # never waits on the exp->mask chain at a boundary.
                wt_prev = emit_score(0)
                if pending_accum is not None:
                    pending_accum()
                    pending_accum = None
                for c in range(1, nch):
                    wt_c = emit_score(c)
                    emit_accum(c - 1, wt_prev)
                    wt_prev = wt_c
                    if c == 1 and pending_tail is not None:
                        pending_tail()
                        pending_tail = None
                    c_head = 1 if nch == 2 else max(2, nch - 4)
                    if c == c_head and heads:
                        emit_head(heads.pop(0))
                pending_accum = (
                    lambda pu=psu, pr=psr, n=nch, w=wt_prev: emit_last_accum(
                        pu, pr, n, w
                    )
                )
                if pending_tail is not None:  # nch == 2 case
                    pending_tail()
                pending_tail = make_tail(i, psu, psr, final=oi == NQB - 1)
            pending_accum()
            pending_tail()

    nc.compile()
    return nc


_NC_CACHE = {}


def _get_nc():
    if "nc" not in _NC_CACHE:
        _NC_CACHE["nc"] = build_kernel()
    return _NC_CACHE["nc"]


def _core_inputs(xb, Wq, Wk, Wv, h):
    """Build the input map for one core (batch data xb [T,C], parity h)."""
    rows = np.concatenate(
        [np.arange(j * P, (j + 1) * P) for j in range(h, T // P, 2)]
    )
    xk = np.ascontiguousarray(xb[rows])            # [NCH*P, C]
    s = np.arange(P)[:, None]
    q = np.arange(QB)[None, :]
    mask_lo = (q >= s + P * h).astype(np.float32)
    mask_hi = (q >= s + P * (h + 2)).astype(np.float32)
    return {
        "xT": np.ascontiguousarray(xb.T),
        "xkT": np.ascontiguousarray(xk.T),
        "xk": xk,
        "wqk": np.ascontiguousarray(Wq.T @ Wk),
        "wv_t": np.ascontiguousarray(Wv.T),
        "mask_lo": mask_lo,
        "mask_hi": mask_hi,
        "ones": np.ones((P, 1), dtype=np.float32),
    }


def kernel(x, Wq, Wk, Wv, _trace=False):
    x = np.asarray(x, dtype=np.float32)
    Wq = np.asarray(Wq, dtype=np.float32)
    Wk = np.asarray(Wk, dtype=np.float32)
    Wv = np.asarray(Wv, dtype=np.float32)

    nc = _get_nc()
    in_maps = [_core_inputs(x[c // 2], Wq, Wk, Wv, c % 2) for c in range(8)]
    try:
        res = run_bass_kernel_spmd(
            nc, in_maps, core_ids=list(range(8)), trace=_trace
        )
    except ModuleNotFoundError:
        # axon NTFF profiling hook unavailable in this container
        res = run_bass_kernel_spmd(nc, in_maps, core_ids=list(range(8)))
    if _trace:
        _NC_CACHE["last_results"] = res

    out = np.empty((B, T, C), dtype=np.float32)
    for b in range(B):
        a, bb = res.results[2 * b], res.results[2 * b + 1]
        denom = a["se"].reshape(T) + bb["se"].reshape(T)
        out[b] = ((a["ou"] + bb["ou"]) / denom[None, :]).T
    return out
